# revision 14
# baseline (speedup 1.0000x reference)
"""GCN layer kernel for 8 Trainium2 NeuronCores.

Computes out = segment_sum(edge_weight * (x @ W + b)[src], dst) for a fixed
problem size: 100000 nodes, 1.6M edges, 512 -> 32 features.

Strategy (v2.2)
---------------
Phase 1 (per core): core c owns nodes [12500c, 12500(c+1)). The host
ships its x slice pre-transposed AND pre-cast to bf16 ([512, 12544]),
halving the dominant HBM read; the loads rotate across the three
DMA-capable queues (SP/ACT/Pool) since one queue tops out at ~22 GB/s
(measured p1: 1188us -> 255us). h = x @ W + b runs on the PE in bf16
with f32 PSUM and lands in a local DRAM table h_local [12544, 32] bf16.

AllGather (bf16, 0.8MB per core): the eight h slices form hg [100352, 32]
bf16 on every core — viewed by the gather as 25088 elements of 256B, each
holding FOUR h rows.

Phase 2 (per core): edges are routed to the core owning their dst. dst
nodes are sorted by in-degree and packed into 98 tiles of 128 "slots";
chunk k holds at most one edge per slot. Per chunk, dma_gather (int16
element indices = h_row//4, 256B elements) pulls [128 slots, 128 bf16]
(4 candidate rows); a DVE multiply against parity-masked bf16 weights
(w at lane quarter h_row%4, zero elsewhere) scales the right row and
zeroes the other three; the PE accumulates chunks into a per-tile PSUM
acc [128, 128] f32 with an identity-stationary bf16 matmul (moving
operand = the masked chunk). Gathers are spread over 4 SWDGE queues
(2048 descriptors each) to parallelize descriptor drain. The accs are staged to SBUF by the ACT
engine and written out unfolded; the host sums the four lane-quarters
while undoing the rank permutation.

kernel() is self-contained: it takes the full inputs, shards them, compiles
the Bass program once (cached), runs it on cores 0-7 and reassembles the
full [100000, 32] output.
"""
import os

import numpy as np
import ml_dtypes

import concourse.bacc as bacc
import concourse.bass as bass
import concourse.tile as tile
from concourse import bass_utils, mybir
from concourse.masks import make_identity
from concourse.tile import add_dep_helper

# ---- problem constants -------------------------------------------------
N_NODES = 100000
N_EDGES = 1600000
IN_F = 512
OUT_F = 32

N_CORES = 8
SHARD_RAW = 12500          # real nodes per core
SHARD = 12544              # padded: 98 tiles of 128
NTILES = SHARD // 128      # 98
TBL_ROWS = N_CORES * SHARD           # 100352 h rows
TBL_ELEMS = TBL_ROWS // 4            # 25088 256B gather elements
KTILES = IN_F // 128       # 4 k-blocks in phase 1

CHUNKS_PER_GROUP = 192     # chunk budget per tile group (SBUF bound)
CHUNKS_PER_GATHER = int(os.environ.get("KM_GCHUNK", "16"))
MAX_TILES_PER_GROUP = 12
SWDGE_SCRATCH = 65536      # SWDGE descriptor ring carveout
N_SWDGE_QUEUES = int(os.environ.get("KM_QUEUES", "4"))


# ---- host prep ---------------------------------------------------------

def _h_row(src):
    return (src // SHARD_RAW) * SHARD + (src % SHARD_RAW)


def prepare(edge_index, edge_weight):
    dst = np.asarray(edge_index[0], dtype=np.int64)
    src = np.asarray(edge_index[1], dtype=np.int64)
    w = np.asarray(edge_weight, dtype=np.float32)

    hr = _h_row(src)
    elem = hr // 4
    par = hr % 4
    core = dst // SHARD_RAW
    dst_local = dst % SHARD_RAW

    deg = np.zeros((N_CORES, SHARD), np.int64)
    np.add.at(deg, (core, dst_local), 1)

    order = np.argsort(-deg, axis=1, kind="stable").astype(np.int32)
    rank = np.empty_like(order)
    np.put_along_axis(
        rank, order, np.arange(SHARD, dtype=np.int32)[None, :], axis=1)

    deg_sorted = np.take_along_axis(deg, order.astype(np.int64), axis=1)
    tile_max = deg_sorted.reshape(N_CORES, NTILES, 128)[:, :, 0]
    K = np.maximum(tile_max.max(axis=0), 1).astype(np.int64)   # [T]

    # tile groups bounded by chunk count (SBUF) and tile count (PSUM)
    groups = []
    t0 = 0
    while t0 < NTILES:
        t1 = t0
        n = 0
        while (t1 < NTILES and t1 - t0 < MAX_TILES_PER_GROUP
               and (n == 0 or n + K[t1] <= CHUNKS_PER_GROUP)):
            n += int(K[t1])
            t1 += 1
        groups.append((t0, t1))
        t0 = t1

    chunk_base = np.zeros(NTILES, np.int64)
    nxt = 0
    for (t0, t1) in groups:
        for t in range(t0, t1):
            chunk_base[t] = nxt
            nxt += int(K[t])
    total_chunks = nxt
    total_slots = total_chunks * 128

    # per-edge slot position
    e_rank = rank[core, dst_local].astype(np.int64)
    e_tile = e_rank // 128
    e_slot = e_rank % 128
    key = core * SHARD + dst_local
    sort_idx = np.argsort(key, kind="stable")
    key_sorted = key[sort_idx]
    first = np.ones(len(key_sorted), bool)
    first[1:] = key_sorted[1:] != key_sorted[:-1]
    run_start = np.maximum.accumulate(
        np.where(first, np.arange(len(key_sorted)), 0))
    k_sorted = np.arange(len(key_sorted)) - run_start
    e_k = np.empty(len(key), np.int64)
    e_k[sort_idx] = k_sorted

    e_pos = (chunk_base[e_tile] + e_k) * 128 + e_slot

    idx_flat = np.zeros((N_CORES, total_slots), np.int16)
    wm_flat = np.zeros((N_CORES, total_slots, 4), ml_dtypes.bfloat16)
    idx_flat[core, e_pos] = elem.astype(np.int16)
    wm_flat[core, e_pos, par] = w

    # gather instruction meta per group: (idx col offset, n chunks)
    gather_meta = []
    icol = 0
    for (t0, t1) in groups:
        nch = int(K[t0:t1].sum())
        gather_meta.append((icol, nch))
        icol += (nch * 128) // 16

    # idx wrapped by 16, replicated to 128 partitions (dma_gather layout)
    idx_wrapped = (idx_flat.reshape(N_CORES, total_slots // 16, 16)
                   .transpose(0, 2, 1))                        # [C, 16, S/16]
    idx_in = np.ascontiguousarray(
        np.tile(idx_wrapped, (1, 8, 1)))                       # [C, 128, S/16]

    # masked weights: [C, 128 slots, total_chunks*4] bf16 (slot-partition)
    wm_in = np.ascontiguousarray(
        wm_flat.reshape(N_CORES, total_chunks, 128, 4)
        .transpose(0, 2, 1, 3)
        .reshape(N_CORES, 128, total_chunks * 4))

    return dict(K=K, groups=groups, total_chunks=total_chunks,
                total_slots=total_slots, gather_meta=gather_meta,
                idx_in=idx_in, wm_in=wm_in, order=order)


# ---- device program ----------------------------------------------------

def build_nc(K, groups, gather_meta, total_chunks, total_slots, reps=0):
    """reps=0: plain program. reps>=1: computation wrapped in a For_i loop
    of `reps` iterations for timing (collective excluded from the loop)."""
    nc = bacc.Bacc("TRN2", target_bir_lowering=False, debug=False,
                   num_devices=N_CORES,
                   dynamic_dma_scratch_size=SWDGE_SCRATCH,
                   num_swdge_queues=N_SWDGE_QUEUES)
    f32 = mybir.dt.float32
    bf16 = mybir.dt.bfloat16
    x_in = nc.dram_tensor("x_sh", [IN_F, SHARD], bf16, kind="ExternalInput")
    wgt_in = nc.dram_tensor("wgt", [IN_F, OUT_F], bf16, kind="ExternalInput")
    bias_in = nc.dram_tensor("bias_r", [128, OUT_F], f32, kind="ExternalInput")
    idx_in = nc.dram_tensor("idx", [128, total_slots // 16], mybir.dt.int16,
                            kind="ExternalInput")
    wm_in = nc.dram_tensor("wmask", [128, total_chunks * 4], bf16,
                           kind="ExternalInput")
    partial = nc.dram_tensor("partial", [NTILES, 128, 128], f32,
                             kind="ExternalOutput")

    with tile.TileContext(nc) as tc:
        with tc.tile_pool(name="dram", bufs=1, space="DRAM") as dram, \
             tc.tile_pool(name="const", bufs=1) as const:
            ident = const.tile([128, 128], f32)
            make_identity(nc, ident[:])
            ident_bf = const.tile([128, 128], bf16)
            nc.vector.tensor_copy(ident_bf[:], ident[:])
            wgt_f = const.tile([128, KTILES, OUT_F], bf16)
            for j in range(KTILES):
                nc.sync.dma_start(wgt_f[:, j, :],
                                  wgt_in[128 * j:128 * (j + 1), :])
            bias_sb = const.tile([128, OUT_F], f32)
            nc.sync.dma_start(bias_sb[:], bias_in[:])

            h_local = dram.tile([SHARD, OUT_F], bf16)
            hg = dram.tile([TBL_ROWS, OUT_F], bf16, addr_space="Shared")

            with tc.tile_pool(name="p1sbuf", bufs=3) as p1s, \
                 tc.tile_pool(name="p1bf", bufs=2) as p1b, \
                 tc.tile_pool(name="p1psum", bufs=2, space="PSUM") as p1p, \
                 tc.tile_pool(name="p1out", bufs=2) as p1o, \
                 tc.tile_pool(name="p2idx", bufs=3) as p2i, \
                 tc.tile_pool(name="p2g", bufs=2) as p2g, \
                 tc.tile_pool(name="p2w", bufs=3) as p2w, \
                 tc.tile_pool(name="p2psum", bufs=2, space="PSUM") as p2p, \
                 tc.tile_pool(name="p2out", bufs=3) as p2o:
                _build_all(nc, tc, K, groups, gather_meta, total_chunks,
                           x_in, idx_in, wm_in, partial, ident, ident_bf,
                           wgt_f, bias_sb, h_local, hg,
                           (p1s, p1b, p1p, p1o, p2i, p2g, p2w, p2p, p2o),
                           reps)
    nc.compile()
    return nc


def _build_all(nc, tc, K, groups, gather_meta, total_chunks,
               x_in, idx_in, wm_in, partial, ident, ident_bf, wgt_f, bias_sb,
               h_local, hg, pools, reps):
    args = (nc, tc, K, groups, gather_meta, total_chunks,
            x_in, idx_in, wm_in, partial, ident, ident_bf, wgt_f, bias_sb,
            h_local, hg, pools)
    if not reps:
        only = os.environ.get("KM_ONLY") or None
        if only == "p2":
            _build_body(*args, only="p2", skip_cc=True)
        elif only:
            _build_body(*args, only=only)
        else:
            _build_body(*args)
    else:
        mode = os.environ.get("KM_LOOP_MODE", "both")
        _build_body(*args, only="p1")
        _build_body(*args, only="cc")
        with tc.For_i(0, reps, 1):
            if mode == "p1":
                _build_body(*args, only="p1")
            elif mode == "p2":
                _build_body(*args, only="p2", skip_cc=True)
            else:
                _build_body(*args, skip_cc=True)


def _build_body(nc, tc, K, groups, gather_meta, total_chunks,
                x_in, idx_in, wm_in, partial, ident, ident_bf, wgt_f, bias_sb,
                h_local, hg, pools, only=None, skip_cc=False):
    f32 = mybir.dt.float32
    bf16 = mybir.dt.bfloat16
    p1s, p1b, p1p, p1o, p2i, p2g, p2w, p2p, p2o = pools

    if only in (None, "p1"):
        # ---------------- phase 1: h = x @ W + b ----------------
        # x arrives pre-transposed from the host ([IN_F, SHARD]); the PE
        # consumes 128-node column blocks directly as the stationary
        # operand — no on-chip transpose stage.
        p1skip = set((os.environ.get("KM_P1SKIP") or "").split(","))
        with nc.named_scope("phase1"):
            for g in range(0, NTILES, 4):
                gt = min(4, NTILES - g)
                xt = p1s.tile([128, KTILES, gt * 128], bf16, tag="xt")
                x_eng = {"gpsimd": nc.gpsimd, "sync": nc.sync,
                         "scalar": nc.scalar, "multi": None}.get(
                    os.environ.get("KM_XDMA", "multi"), None)
                for j in ([] if "dma" in p1skip else range(KTILES)):
                    # one DMA queue tops out at ~22 GB/s and the x load
                    # dominates phase 1 — balance bytes evenly across
                    # the SP/ACT/Pool queues with a global rotation
                    # ((g//4)*KTILES + j carries no 4-cycle bias).
                    eng = ([nc.sync, nc.scalar, nc.gpsimd]
                           [((g // 4) * KTILES + j) % 3]
                           if x_eng is None else x_eng)
                    eng.dma_start(
                        xt[:, j, :],
                        x_in[128 * j:128 * (j + 1),
                             128 * g:128 * (g + gt)])
                hb = p1o.tile([128, 4, OUT_F], bf16, tag="hb")
                for i in ([] if "compute" in p1skip else range(gt)):
                    h_ps = p1p.tile([128, OUT_F], f32, space="PSUM",
                                    tag="h_ps")
                    for j in range(KTILES):
                        nc.tensor.matmul(
                            out=h_ps[:],
                            lhsT=xt[:, j, 128 * i:128 * (i + 1)],
                            rhs=wgt_f[:, j, :],
                            start=(j == 0), stop=(j == KTILES - 1))
                    nc.vector.tensor_tensor(
                        out=hb[:, i, :], in0=h_ps[:],
                        in1=bias_sb[:],
                        op=mybir.AluOpType.add)
                if "out" not in p1skip:
                    nc.scalar.dma_start(
                        h_local[128 * g:128 * (g + gt), :]
                        .rearrange("(a p) f -> p a f", p=128),
                        hb[:, :gt, :])

    if only == "p1":
        return
    if only in (None, "cc") and not skip_cc:
        # ---------------- allgather (bf16, 0.8MB per core) -------
        # NOTE: hg is Shared DRAM -> exactly ONE writer instruction is
        # allowed, so the collective cannot be split/overlapped with p1.
        with nc.named_scope("allgather"):
            ccs = [nc.gpsimd.collective_compute(
                "AllGather", mybir.AluOpType.bypass,
                replica_groups=[list(range(N_CORES))],
                ins=[h_local[:].opt()], outs=[hg[:].opt()])]
    else:
        ccs = []
    if only == "cc":
        return

    # ---------------- phase 2: gather + mask-scale + accumulate ----
    skip = set((os.environ.get("KM_SKIP") or "").split(","))
    with nc.named_scope("phase2"):
        # hg viewed as 25088 x 128 bf16 elements (4 rows per element)
        hg_e = hg[:].rearrange("(e q) f -> e (q f)", q=4)
        chunk_off = 0
        for gi, (t0, t1) in enumerate(groups):
            icol0, nch = gather_meta[gi]
            icols = (nch * 128) // 16
            idx_sb = p2i.tile([128, icols], mybir.dt.int16, tag="idx")
            nc.sync.dma_start(idx_sb[:], idx_in[:, icol0:icol0 + icols])
            wm_sb = p2w.tile([128, nch * 4], bf16, tag="wm")
            # idx rides the SP queue; wm rides ACT so the two per-group
            # table loads drain in parallel ahead of the gathers.
            nc.scalar.dma_start(
                wm_sb[:], wm_in[:, chunk_off * 4:(chunk_off + nch) * 4])

            g_sb = p2g.tile([128, nch, 128], bf16, tag="g", name=f"g{gi}")
            for b in ([] if "gather" in skip else
                      range(0, nch, CHUNKS_PER_GATHER)):
                c1 = min(nch, b + CHUNKS_PER_GATHER)
                ni = (c1 - b) * 128
                gi_inst = nc.gpsimd.dma_gather(
                    out_ap=g_sb[:, b:c1, :],
                    in_ap=hg_e,
                    idxs_ap=idx_sb[:, b * 8:b * 8 + ni // 16],
                    num_idxs=ni,
                    num_idxs_reg=ni,
                    elem_size=128,
                    queue_num=(b // CHUNKS_PER_GATHER) % N_SWDGE_QUEUES,
                    single_packet=False)
                for cc in ccs:
                    add_dep_helper(gi_inst.ins, cc.ins,
                                   reason="gather reads hg")

            # mask-scale: g[p, c, 32q+f] *= wm[p, c, q]  (zero off-parity)
            g_flat = g_sb[:].rearrange("p c v -> p (c v)") \
                .rearrange("p (m f) -> p m f", f=OUT_F)       # [128,4nch,32]
            if "mult" not in skip:
                nc.vector.tensor_tensor(
                    out=g_flat,
                    in0=g_flat,
                    in1=wm_sb[:].unsqueeze(2)
                    .to_broadcast([128, nch * 4, OUT_F]),
                    op=mybir.AluOpType.mult)

            # accumulate chunks into per-tile PSUM via identity-stationary
            accs = []
            kk = 0
            for t in range(t0, t1):
                acc = p2p.tile([128, 128], f32, space="PSUM", tag="acc",
                               name=f"acc{gi}_{t}")
                for k in ([] if "pe" in skip else range(int(K[t]))):
                    nc.tensor.matmul(
                        out=acc[:],
                        lhsT=ident_bf[:],
                        rhs=g_sb[:, kk, :],
                        start=(k == 0), stop=(k == int(K[t]) - 1))
                    kk += 1
                accs.append(acc)

            # stage the unfolded accs via the (otherwise idle) ACT engine;
            # the lane-quarter fold happens on the host in combine2().
            if "out" not in skip:
                ob = p2o.tile([128, t1 - t0, 128], f32, tag="ob")
                for i in range(t1 - t0):
                    nc.scalar.copy(ob[:, i, :], accs[i][:])
                nc.sync.dma_start(
                    partial[t0:t1].rearrange("t p f -> p t f"), ob[:])
            chunk_off += nch


# ---- output combination ------------------------------------------------

def combine2(partials, prep):
    """partials: list per core of [NTILES, 128, 128] (rank-ordered rows,
    unfolded lane-quarters)."""
    order = prep["order"]
    out = np.empty((N_CORES * SHARD_RAW, OUT_F), np.float32)
    for c in range(N_CORES):
        p = np.asarray(partials[c]).reshape(SHARD, 4, OUT_F).sum(axis=1)
        rank = np.empty(SHARD, np.int64)
        rank[np.asarray(order[c], np.int64)] = np.arange(SHARD)
        out[c * SHARD_RAW:(c + 1) * SHARD_RAW] = p[rank[:SHARD_RAW]]
    return out


# ---- entry point -------------------------------------------------------

_CACHE = {}


def kernel(x, weight, bias, edge_weight, edge_index):
    x = np.asarray(x, np.float32)
    weight = np.asarray(weight, np.float32)
    bias = np.asarray(bias, np.float32)
    edge_weight = np.asarray(edge_weight, np.float32)
    edge_index = np.asarray(edge_index, np.int32)

    prep = prepare(edge_index, edge_weight)

    key = (tuple(prep["K"]), tuple(prep["groups"]))
    if key not in _CACHE:
        _CACHE.clear()
        _CACHE[key] = build_nc(prep["K"], prep["groups"], prep["gather_meta"],
                               prep["total_chunks"], prep["total_slots"])
    nc = _CACHE[key]

    x_pad = np.zeros((N_CORES, IN_F, SHARD), ml_dtypes.bfloat16)
    for c in range(N_CORES):
        n0 = c * SHARD_RAW
        x_pad[c, :, :SHARD_RAW] = x[n0:n0 + SHARD_RAW].T.astype(
            ml_dtypes.bfloat16)

    in_maps = [{
        "x_sh": x_pad[c],
        "wgt": weight.astype(ml_dtypes.bfloat16),
        "bias_r": np.ascontiguousarray(
            np.broadcast_to(bias.reshape(1, OUT_F), (128, OUT_F))),
        "idx": prep["idx_in"][c],
        "wmask": prep["wm_in"][c],
    } for c in range(N_CORES)]

    trace = bool(os.environ.get("KM_TRACE"))
    kw = {}
    if trace:
        kw = dict(trace=True, trace_cores=list(range(N_CORES)),
                  stitch_traces=bool(os.environ.get("KM_STITCH")))
    res = bass_utils.run_bass_kernel_spmd(
        nc, in_maps, core_ids=list(range(N_CORES)), **kw)
    global LAST_RESULTS
    LAST_RESULTS = res
    partials = [r["partial"] for r in res.results]
    return combine2(partials, prep)


LAST_RESULTS = None

X_TRANSPOSED = True



# revision 15
# speedup vs baseline: 1.1333x; 1.1333x over previous
"""GCN layer kernel for 8 Trainium2 NeuronCores.

Computes out = segment_sum(edge_weight * (x @ W + b)[src], dst) for a fixed
problem size: 100000 nodes, 1.6M edges, 512 -> 32 features.

Strategy (v2.2)
---------------
Phase 1 (per core): core c owns nodes [12500c, 12500(c+1)). The host
ships its x slice pre-transposed AND pre-cast to bf16 ([512, 12544]),
halving the dominant HBM read; the loads rotate across the three
DMA-capable queues (SP/ACT/Pool) since one queue tops out at ~22 GB/s
(measured p1: 1188us -> 156us). h = x @ W + b runs on the PE in bf16
with f32 PSUM and lands in a local DRAM table h_local [12544, 32] bf16.

AllGather (bf16, 0.8MB per core): the eight h slices form hg [100352, 32]
bf16 on every core — viewed by the gather as 25088 elements of 256B, each
holding FOUR h rows.

Phase 2 (per core): edges are routed to the core owning their dst. dst
nodes are sorted by in-degree and packed into 98 tiles of 128 "slots";
chunk k holds at most one edge per slot. Per chunk, dma_gather (int16
element indices = h_row//4, 256B elements) pulls [128 slots, 128 bf16]
(4 candidate rows); a DVE multiply against parity-masked bf16 weights
(w at lane quarter h_row%4, zero elsewhere) scales the right row and
zeroes the other three; the PE accumulates chunks into a per-tile PSUM
acc [128, 128] f32 with an identity-stationary bf16 matmul (moving
operand = the masked chunk). Gathers are spread over 4 SWDGE queues
(2048 descriptors each) to parallelize descriptor drain. The accs are staged to SBUF by the ACT
engine and written out unfolded; the host sums the four lane-quarters
while undoing the rank permutation.

kernel() is self-contained: it takes the full inputs, shards them, compiles
the Bass program once (cached), runs it on cores 0-7 and reassembles the
full [100000, 32] output.
"""
import os

import numpy as np
import ml_dtypes

import concourse.bacc as bacc
import concourse.bass as bass
import concourse.tile as tile
from concourse import bass_utils, mybir
from concourse.masks import make_identity
from concourse.tile import add_dep_helper

# ---- problem constants -------------------------------------------------
N_NODES = 100000
N_EDGES = 1600000
IN_F = 512
OUT_F = 32

N_CORES = 8
SHARD_RAW = 12500          # real nodes per core
SHARD = 12544              # padded: 98 tiles of 128
NTILES = SHARD // 128      # 98
TBL_ROWS = N_CORES * SHARD           # 100352 h rows
TBL_ELEMS = TBL_ROWS // 4            # 25088 256B gather elements
KTILES = IN_F // 128       # 4 k-blocks in phase 1

CHUNKS_PER_GROUP = 192     # chunk budget per tile group (SBUF bound)
CHUNKS_PER_GATHER = int(os.environ.get("KM_GCHUNK", "16"))
MAX_TILES_PER_GROUP = 12
SWDGE_SCRATCH = 65536      # SWDGE descriptor ring carveout
N_SWDGE_QUEUES = int(os.environ.get("KM_QUEUES", "4"))


# ---- host prep ---------------------------------------------------------

def _h_row(src):
    return (src // SHARD_RAW) * SHARD + (src % SHARD_RAW)


def prepare(edge_index, edge_weight):
    dst = np.asarray(edge_index[0], dtype=np.int64)
    src = np.asarray(edge_index[1], dtype=np.int64)
    w = np.asarray(edge_weight, dtype=np.float32)

    hr = _h_row(src)
    elem = hr // 4
    par = hr % 4
    core = dst // SHARD_RAW
    dst_local = dst % SHARD_RAW

    deg = np.zeros((N_CORES, SHARD), np.int64)
    np.add.at(deg, (core, dst_local), 1)

    order = np.argsort(-deg, axis=1, kind="stable").astype(np.int32)
    rank = np.empty_like(order)
    np.put_along_axis(
        rank, order, np.arange(SHARD, dtype=np.int32)[None, :], axis=1)

    deg_sorted = np.take_along_axis(deg, order.astype(np.int64), axis=1)
    tile_max = deg_sorted.reshape(N_CORES, NTILES, 128)[:, :, 0]
    K = np.maximum(tile_max.max(axis=0), 1).astype(np.int64)   # [T]

    # tile groups bounded by chunk count (SBUF) and tile count (PSUM)
    groups = []
    t0 = 0
    while t0 < NTILES:
        t1 = t0
        n = 0
        while (t1 < NTILES and t1 - t0 < MAX_TILES_PER_GROUP
               and (n == 0 or n + K[t1] <= CHUNKS_PER_GROUP)):
            n += int(K[t1])
            t1 += 1
        groups.append((t0, t1))
        t0 = t1

    chunk_base = np.zeros(NTILES, np.int64)
    nxt = 0
    for (t0, t1) in groups:
        for t in range(t0, t1):
            chunk_base[t] = nxt
            nxt += int(K[t])
    total_chunks = nxt
    total_slots = total_chunks * 128

    # per-edge slot position
    e_rank = rank[core, dst_local].astype(np.int64)
    e_tile = e_rank // 128
    e_slot = e_rank % 128
    key = core * SHARD + dst_local
    sort_idx = np.argsort(key, kind="stable")
    key_sorted = key[sort_idx]
    first = np.ones(len(key_sorted), bool)
    first[1:] = key_sorted[1:] != key_sorted[:-1]
    run_start = np.maximum.accumulate(
        np.where(first, np.arange(len(key_sorted)), 0))
    k_sorted = np.arange(len(key_sorted)) - run_start
    e_k = np.empty(len(key), np.int64)
    e_k[sort_idx] = k_sorted

    e_pos = (chunk_base[e_tile] + e_k) * 128 + e_slot

    idx_flat = np.zeros((N_CORES, total_slots), np.int16)
    wm_flat = np.zeros((N_CORES, total_slots, 4), ml_dtypes.bfloat16)
    idx_flat[core, e_pos] = elem.astype(np.int16)
    wm_flat[core, e_pos, par] = w

    # gather instruction meta per group: (idx col offset, n chunks)
    gather_meta = []
    icol = 0
    for (t0, t1) in groups:
        nch = int(K[t0:t1].sum())
        gather_meta.append((icol, nch))
        icol += (nch * 128) // 16

    # idx wrapped by 16, replicated to 128 partitions (dma_gather layout)
    idx_wrapped = (idx_flat.reshape(N_CORES, total_slots // 16, 16)
                   .transpose(0, 2, 1))                        # [C, 16, S/16]
    idx_in = np.ascontiguousarray(
        np.tile(idx_wrapped, (1, 8, 1)))                       # [C, 128, S/16]

    # masked weights: [C, 128 slots, total_chunks*4] bf16 (slot-partition)
    wm_in = np.ascontiguousarray(
        wm_flat.reshape(N_CORES, total_chunks, 128, 4)
        .transpose(0, 2, 1, 3)
        .reshape(N_CORES, 128, total_chunks * 4))

    return dict(K=K, groups=groups, total_chunks=total_chunks,
                total_slots=total_slots, gather_meta=gather_meta,
                idx_in=idx_in, wm_in=wm_in, order=order)


# ---- device program ----------------------------------------------------

def build_nc(K, groups, gather_meta, total_chunks, total_slots, reps=0):
    """reps=0: plain program. reps>=1: computation wrapped in a For_i loop
    of `reps` iterations for timing (collective excluded from the loop)."""
    nc = bacc.Bacc("TRN2", target_bir_lowering=False, debug=False,
                   num_devices=N_CORES,
                   dynamic_dma_scratch_size=SWDGE_SCRATCH,
                   num_swdge_queues=N_SWDGE_QUEUES)
    f32 = mybir.dt.float32
    bf16 = mybir.dt.bfloat16
    x_in = nc.dram_tensor("x_sh", [IN_F, SHARD], bf16, kind="ExternalInput")
    wgt_in = nc.dram_tensor("wgt", [IN_F, OUT_F], bf16, kind="ExternalInput")
    bias_in = nc.dram_tensor("bias_r", [128, OUT_F], f32, kind="ExternalInput")
    idx_in = nc.dram_tensor("idx", [128, total_slots // 16], mybir.dt.int16,
                            kind="ExternalInput")
    wm_in = nc.dram_tensor("wmask", [128, total_chunks * 4], bf16,
                           kind="ExternalInput")
    partial = nc.dram_tensor("partial", [NTILES, 128, 128], f32,
                             kind="ExternalOutput")

    with tile.TileContext(nc) as tc:
        with tc.tile_pool(name="dram", bufs=1, space="DRAM") as dram, \
             tc.tile_pool(name="const", bufs=1) as const:
            ident = const.tile([128, 128], f32)
            make_identity(nc, ident[:])
            ident_bf = const.tile([128, 128], bf16)
            nc.vector.tensor_copy(ident_bf[:], ident[:])
            wgt_f = const.tile([128, KTILES, OUT_F], bf16)
            for j in range(KTILES):
                nc.sync.dma_start(wgt_f[:, j, :],
                                  wgt_in[128 * j:128 * (j + 1), :])
            bias_sb = const.tile([128, OUT_F], f32)
            nc.sync.dma_start(bias_sb[:], bias_in[:])

            h_local = dram.tile([SHARD, OUT_F], bf16)
            hg = dram.tile([TBL_ROWS, OUT_F], bf16, addr_space="Shared")

            with tc.tile_pool(name="p1sbuf", bufs=3) as p1s, \
                 tc.tile_pool(name="p1bf", bufs=2) as p1b, \
                 tc.tile_pool(name="p1psum", bufs=2, space="PSUM") as p1p, \
                 tc.tile_pool(name="p1out", bufs=2) as p1o, \
                 tc.tile_pool(name="p2idx", bufs=3) as p2i, \
                 tc.tile_pool(name="p2g", bufs=2) as p2g, \
                 tc.tile_pool(name="p2w", bufs=3) as p2w, \
                 tc.tile_pool(name="p2psum", bufs=2, space="PSUM") as p2p, \
                 tc.tile_pool(name="p2out", bufs=3) as p2o:
                _build_all(nc, tc, K, groups, gather_meta, total_chunks,
                           x_in, idx_in, wm_in, partial, ident, ident_bf,
                           wgt_f, bias_sb, h_local, hg,
                           (p1s, p1b, p1p, p1o, p2i, p2g, p2w, p2p, p2o),
                           reps)
    nc.compile()
    return nc


def _build_all(nc, tc, K, groups, gather_meta, total_chunks,
               x_in, idx_in, wm_in, partial, ident, ident_bf, wgt_f, bias_sb,
               h_local, hg, pools, reps):
    args = (nc, tc, K, groups, gather_meta, total_chunks,
            x_in, idx_in, wm_in, partial, ident, ident_bf, wgt_f, bias_sb,
            h_local, hg, pools)
    if not reps:
        only = os.environ.get("KM_ONLY") or None
        if only == "p2":
            _build_body(*args, only="p2", skip_cc=True)
        elif only:
            _build_body(*args, only=only)
        else:
            _build_body(*args)
    else:
        mode = os.environ.get("KM_LOOP_MODE", "both")
        _build_body(*args, only="p1")
        _build_body(*args, only="cc")
        with tc.For_i(0, reps, 1):
            if mode == "p1":
                _build_body(*args, only="p1")
            elif mode == "p2":
                _build_body(*args, only="p2", skip_cc=True)
            else:
                _build_body(*args, skip_cc=True)


def _build_body(nc, tc, K, groups, gather_meta, total_chunks,
                x_in, idx_in, wm_in, partial, ident, ident_bf, wgt_f, bias_sb,
                h_local, hg, pools, only=None, skip_cc=False):
    f32 = mybir.dt.float32
    bf16 = mybir.dt.bfloat16
    p1s, p1b, p1p, p1o, p2i, p2g, p2w, p2p, p2o = pools

    if only in (None, "p1"):
        # ---------------- phase 1: h = x @ W + b ----------------
        # x arrives pre-transposed from the host ([IN_F, SHARD]); the PE
        # consumes 128-node column blocks directly as the stationary
        # operand — no on-chip transpose stage.
        p1skip = set((os.environ.get("KM_P1SKIP") or "").split(","))
        with nc.named_scope("phase1"):
            for g in range(0, NTILES, 4):
                gt = min(4, NTILES - g)
                xt = p1s.tile([128, KTILES, gt * 128], bf16, tag="xt")
                x_eng = {"gpsimd": nc.gpsimd, "sync": nc.sync,
                         "scalar": nc.scalar, "multi": None}.get(
                    os.environ.get("KM_XDMA", "multi"), None)
                for j in ([] if "dma" in p1skip else range(KTILES)):
                    # one DMA queue tops out at ~22 GB/s and the x load
                    # dominates phase 1 — balance bytes evenly across
                    # the SP/ACT/Pool queues with a global rotation
                    # ((g//4)*KTILES + j carries no 4-cycle bias).
                    eng = ([nc.sync, nc.scalar, nc.gpsimd]
                           [((g // 4) * KTILES + j) % 3]
                           if x_eng is None else x_eng)
                    eng.dma_start(
                        xt[:, j, :],
                        x_in[128 * j:128 * (j + 1),
                             128 * g:128 * (g + gt)])
                hb = p1o.tile([128, 4, OUT_F], bf16, tag="hb")
                for i in ([] if "compute" in p1skip else range(gt)):
                    h_ps = p1p.tile([128, OUT_F], f32, space="PSUM",
                                    tag="h_ps")
                    for j in range(KTILES):
                        nc.tensor.matmul(
                            out=h_ps[:],
                            lhsT=xt[:, j, 128 * i:128 * (i + 1)],
                            rhs=wgt_f[:, j, :],
                            start=(j == 0), stop=(j == KTILES - 1))
                    nc.vector.tensor_tensor(
                        out=hb[:, i, :], in0=h_ps[:],
                        in1=bias_sb[:],
                        op=mybir.AluOpType.add)
                if "out" not in p1skip:
                    nc.scalar.dma_start(
                        h_local[128 * g:128 * (g + gt), :]
                        .rearrange("(a p) f -> p a f", p=128),
                        hb[:, :gt, :])

    if only == "p1":
        return
    if only in (None, "cc") and not skip_cc:
        # ---------------- allgather (bf16, 0.8MB per core) -------
        # NOTE: hg is Shared DRAM -> exactly ONE writer instruction is
        # allowed, so the collective cannot be split/overlapped with p1.
        with nc.named_scope("allgather"):
            ccs = [nc.gpsimd.collective_compute(
                "AllGather", mybir.AluOpType.bypass,
                replica_groups=[list(range(N_CORES))],
                ins=[h_local[:].opt()], outs=[hg[:].opt()])]
    else:
        ccs = []
    if only == "cc":
        return

    # ---------------- phase 2: gather + mask-scale + accumulate ----
    skip = set((os.environ.get("KM_SKIP") or "").split(","))
    with nc.named_scope("phase2"):
        # hg viewed as 25088 x 128 bf16 elements (4 rows per element)
        hg_e = hg[:].rearrange("(e q) f -> e (q f)", q=4)
        chunk_off = 0
        for gi, (t0, t1) in enumerate(groups):
            icol0, nch = gather_meta[gi]
            icols = (nch * 128) // 16
            idx_sb = p2i.tile([128, icols], mybir.dt.int16, tag="idx")
            nc.sync.dma_start(idx_sb[:], idx_in[:, icol0:icol0 + icols])
            wm_sb = p2w.tile([128, nch * 4], bf16, tag="wm")
            # idx rides the SP queue; wm rides ACT so the two per-group
            # table loads drain in parallel ahead of the gathers.
            nc.scalar.dma_start(
                wm_sb[:], wm_in[:, chunk_off * 4:(chunk_off + nch) * 4])

            g_sb = p2g.tile([128, nch, 128], bf16, tag="g", name=f"g{gi}")
            for b in ([] if "gather" in skip else
                      range(0, nch, CHUNKS_PER_GATHER)):
                c1 = min(nch, b + CHUNKS_PER_GATHER)
                ni = (c1 - b) * 128
                gi_inst = nc.gpsimd.dma_gather(
                    out_ap=g_sb[:, b:c1, :],
                    in_ap=hg_e,
                    idxs_ap=idx_sb[:, b * 8:b * 8 + ni // 16],
                    num_idxs=ni,
                    num_idxs_reg=ni,
                    elem_size=128,
                    queue_num=(b // CHUNKS_PER_GATHER) % N_SWDGE_QUEUES,
                    single_packet=False)
                for cc in ccs:
                    add_dep_helper(gi_inst.ins, cc.ins,
                                   reason="gather reads hg")

            # mask-scale: g[p, c, 32q+f] *= wm[p, c, q]  (zero off-parity)
            g_flat = g_sb[:].rearrange("p c v -> p (c v)") \
                .rearrange("p (m f) -> p m f", f=OUT_F)       # [128,4nch,32]
            if "mult" not in skip:
                nc.vector.tensor_tensor(
                    out=g_flat,
                    in0=g_flat,
                    in1=wm_sb[:].unsqueeze(2)
                    .to_broadcast([128, nch * 4, OUT_F]),
                    op=mybir.AluOpType.mult)

            # accumulate chunks into per-tile PSUM via identity-stationary
            accs = []
            kk = 0
            for t in range(t0, t1):
                acc = p2p.tile([128, 128], f32, space="PSUM", tag="acc",
                               name=f"acc{gi}_{t}")
                for k in ([] if "pe" in skip else range(int(K[t]))):
                    nc.tensor.matmul(
                        out=acc[:],
                        lhsT=ident_bf[:],
                        rhs=g_sb[:, kk, :],
                        start=(k == 0), stop=(k == int(K[t]) - 1))
                    kk += 1
                accs.append(acc)

            # stage the unfolded accs via the (otherwise idle) ACT engine;
            # the lane-quarter fold happens on the host in combine2().
            if "out" not in skip:
                ob = p2o.tile([128, t1 - t0, 128], f32, tag="ob")
                for i in range(t1 - t0):
                    nc.scalar.copy(ob[:, i, :], accs[i][:])
                nc.sync.dma_start(
                    partial[t0:t1].rearrange("t p f -> p t f"), ob[:])
            chunk_off += nch


# ---- output combination ------------------------------------------------

def combine2(partials, prep):
    """partials: list per core of [NTILES, 128, 128] (rank-ordered rows,
    unfolded lane-quarters)."""
    order = prep["order"]
    out = np.empty((N_CORES * SHARD_RAW, OUT_F), np.float32)
    for c in range(N_CORES):
        p = np.asarray(partials[c]).reshape(SHARD, 4, OUT_F).sum(axis=1)
        rank = np.empty(SHARD, np.int64)
        rank[np.asarray(order[c], np.int64)] = np.arange(SHARD)
        out[c * SHARD_RAW:(c + 1) * SHARD_RAW] = p[rank[:SHARD_RAW]]
    return out


# ---- entry point -------------------------------------------------------

_CACHE = {}


def kernel(x, weight, bias, edge_weight, edge_index):
    x = np.asarray(x, np.float32)
    weight = np.asarray(weight, np.float32)
    bias = np.asarray(bias, np.float32)
    edge_weight = np.asarray(edge_weight, np.float32)
    edge_index = np.asarray(edge_index, np.int32)

    prep = prepare(edge_index, edge_weight)

    key = (tuple(prep["K"]), tuple(prep["groups"]))
    if key not in _CACHE:
        _CACHE.clear()
        _CACHE[key] = build_nc(prep["K"], prep["groups"], prep["gather_meta"],
                               prep["total_chunks"], prep["total_slots"])
    nc = _CACHE[key]

    x_pad = np.zeros((N_CORES, IN_F, SHARD), ml_dtypes.bfloat16)
    for c in range(N_CORES):
        n0 = c * SHARD_RAW
        x_pad[c, :, :SHARD_RAW] = x[n0:n0 + SHARD_RAW].T.astype(
            ml_dtypes.bfloat16)

    in_maps = [{
        "x_sh": x_pad[c],
        "wgt": weight.astype(ml_dtypes.bfloat16),
        "bias_r": np.ascontiguousarray(
            np.broadcast_to(bias.reshape(1, OUT_F), (128, OUT_F))),
        "idx": prep["idx_in"][c],
        "wmask": prep["wm_in"][c],
    } for c in range(N_CORES)]

    trace = bool(os.environ.get("KM_TRACE"))
    kw = {}
    if trace:
        kw = dict(trace=True, trace_cores=list(range(N_CORES)),
                  stitch_traces=bool(os.environ.get("KM_STITCH")))
    res = bass_utils.run_bass_kernel_spmd(
        nc, in_maps, core_ids=list(range(N_CORES)), **kw)
    global LAST_RESULTS
    LAST_RESULTS = res
    partials = [r["partial"] for r in res.results]
    return combine2(partials, prep)


LAST_RESULTS = None

X_TRANSPOSED = True

